# revision 9
# baseline (speedup 1.0000x reference)
"""DkNN retrieval kernel for 8 trn2 NeuronCores (self-contained).

Algorithm (matches reference.py):
  xq = x/||x|| - center;  score_j = ||X_j||^2 - 2 xq.X_j;  closest = argmin_j
  neigh = [closest, tni[closest]];  counts = bincount(labels[neigh]);
  p = (1000 - bisect_left(cali, 75-counts))/1000;  creds = onehot(argmax p)*max p

Distribution: X sharded over 8 cores on the train axis (12500 rows each,
padded to 12800 with fake rows whose ||X||^2 = 1e4, never winning). Queries
replicated. Matmuls use a 3-term bf16 split (hi*Hi + hi*Lo + lo*Hi) for
~2e-7 score accuracy (the rel-err gate effectively requires zero argmin
flips; one flip costs ~4e-2 rel err).

Structure: scores accumulate into 1024-wide (2-bank) PSUM windows; the two
512-col halves of a window share the stationary query weight back-to-back
(avoids the ~47ns PE weight-switch bubble on half the matmuls). Two custom
DVE ops reduce each window straight out of PSUM (argmin-position scan +
min-value accum over ps+ss). Query tiles are processed in two blocks of 4;
block A's cross-core AllToAll runs in the shadow of block B's matmuls.
Everything query-independent (||X||^2, bf16 splits, query normalization,
neighbor-label bincounts + conformal LUT folded into a per-train-point
p-value table F2[j,c]) is host preprocessing; the tail is one indirect
row-gather of F2 + a short argmax chain.
"""
import os
import numpy as np

import concourse.bass as bass
import concourse.bacc as bacc
import concourse.tile as tile
import concourse.mybir as mybir
import concourse.dve_ops as dve_ops_mod
from concourse.bass_utils import run_bass_kernel_spmd
from concourse.dve_ops import DveOp, OPS
from concourse.dve_spec import Spec, Src0, Src1, C0, MaxNeg, scan, select, eq, Idx, lower
from concourse.dve_uop import DveOpSpec, AluOp
from concourse.dve_table_gen import dve_ver_for

NB_DATA = 1024
NB_TRAIN = 100000
D = 256
NB_CALI = 1000
NCORES = 8

SHARD = 12500          # real candidates per core
SHARD_PAD = 12800      # padded (fake rows score 1e4, never win)
WIN = 1024             # candidate columns per PSUM window (2 banks)
NWIN = 13              # 12 full windows + 1 half (512)
QT = 8                 # query tiles of 128
QBLK = 4               # query tiles per collective block
NCOLB = NWIN * QBLK    # 52 accumulator columns per block

_AluOp = mybir.AluOpType


def _register_op(name, spec_fn):
    if name in dve_ops_mod._SUB_OPCODE_FOR_NAME:
        for op in OPS:
            if op.name == name:
                return op
    spec = spec_fn()
    opcode = dve_ops_mod._CUSTOM_DVE_ROW_BASE + len(OPS)
    dve_ops_mod._SUB_OPCODE_FOR_NAME[name] = opcode
    ver = dve_ver_for("TRN2")
    tmp = DveOpSpec(name=name, opcode=opcode, uops=lower(spec, ver=ver),
                    rd1_en=True)
    op = DveOp(name, spec, subdim=False, uops_sha={ver: tmp.sha(ver)})
    OPS.append(op)
    return op


def _idx_scan_spec():
    s = Src0 + Src1
    r = scan(AluOp.MIN, s, init=C0)
    body = select(eq(s, r), Idx, MaxNeg)

    def ref(in0, in1, s0, s1, imm2):
        v = (in0.astype(np.float64) + in1.astype(np.float64)).astype(np.float32)
        rm = np.minimum(np.minimum.accumulate(v, axis=-1), np.float32(s0))
        idx = np.arange(v.shape[-1], dtype=np.float64)
        sel = np.where(v == rm, idx, -3.4e38)
        return sel.astype(np.float32)

    return Spec(body=body, accum=AluOp.MAX, reference=ref)


def _val_min_spec():
    # accum_out = min over stream of (Src0 + Src1); out stream is junk
    return Spec(body=Src0 + Src1, accum=AluOp.MIN, accum_init=C0)


IDX_SCAN = _register_op("IDX_SCAN_ANT", _idx_scan_spec)
VAL_MIN = _register_op("VAL_MIN_ANT", _val_min_spec)
dt = mybir.dt


def build_kernel():
    nc = bacc.Bacc("TRN2", target_bir_lowering=False, debug=False,
                   num_devices=NCORES)

    # ---- I/O ----
    qh = [nc.dram_tensor(f"qh{k}", [128, NB_DATA], dt.bfloat16,
                         kind="ExternalInput").ap() for k in range(2)]
    ql = [nc.dram_tensor(f"ql{k}", [128, NB_DATA], dt.bfloat16,
                         kind="ExternalInput").ap() for k in range(2)]
    xh = [nc.dram_tensor(f"xh{k}", [128, SHARD_PAD], dt.bfloat16,
                         kind="ExternalInput").ap() for k in range(2)]
    xl = [nc.dram_tensor(f"xl{k}", [128, SHARD_PAD], dt.bfloat16,
                         kind="ExternalInput").ap() for k in range(2)]
    ssg = nc.dram_tensor("ssg", [1, SHARD_PAD], dt.float32, kind="ExternalInput").ap()
    posc = nc.dram_tensor("posc", [128, 2 * NCOLB], dt.float32, kind="ExternalInput").ap()
    f2 = nc.dram_tensor("f2", [NB_TRAIN, 10], dt.float32, kind="ExternalInput").ap()
    io10 = nc.dram_tensor("io10", [128, 10], dt.float32, kind="ExternalInput").ap()
    creds_out = nc.dram_tensor("creds", [128, 10], dt.float32, kind="ExternalOutput").ap()

    with tile.TileContext(nc) as tc:
        with tc.tile_pool(name="dram", bufs=1, space="DRAM") as dpool:
            loc_d = [dpool.tile([NB_DATA // 2, 2], dt.float32, name=f"loc{b}")
                     for b in range(2)]
            glob_d = [dpool.tile([NCORES, 64, 2], dt.float32, name=f"glob{b}")
                      for b in range(2)]

            with tc.tile_pool(name="mp", bufs=1, side="right") as mp, \
                 tc.tile_pool(name="mp2", bufs=2, side="right") as mp2, \
                 tc.tile_pool(name="pp", bufs=1, space="PSUM") as pp:

                # ===== persistent loads (critical-path DMAs first: warmup) =====
                ssrow = mp.tile([1, SHARD_PAD], dt.float32)
                nc.sync.dma_start(ssrow[:], ssg[:, :])
                xht0 = [mp2.tile([128, WIN], dt.bfloat16, tag=f"xht{k}",
                                 bufs=3, name=f"xht{k}_w0") for k in range(2)]
                xlt0 = [mp2.tile([128, WIN], dt.bfloat16, tag=f"xlt{k}",
                                 bufs=3, name=f"xlt{k}_w0") for k in range(2)]
                for k in range(2):
                    nc.sync.dma_start(xht0[k][:], xh[k][:, 0:WIN])
                    nc.sync.dma_start(xlt0[k][:], xl[k][:, 0:WIN])
                qht = [mp.tile([128, NB_DATA], dt.bfloat16, name=f"qht{k}")
                       for k in range(2)]
                qlt = [mp.tile([128, NB_DATA], dt.bfloat16, name=f"qlt{k}")
                       for k in range(2)]
                HQ = NB_DATA // 2
                for k in range(2):  # block-A query halves first
                    nc.sync.dma_start(qht[k][:, 0:HQ], qh[k][:, 0:HQ])
                    nc.sync.dma_start(qlt[k][:, 0:HQ], ql[k][:, 0:HQ])
                ssb = mp.tile([128, SHARD_PAD], dt.float32)
                CH = SHARD_PAD // 4
                for j in range(4):
                    nc.gpsimd.partition_broadcast(
                        ssb[:, j * CH:(j + 1) * CH], ssrow[:, j * CH:(j + 1) * CH])
                for k in range(2):
                    nc.sync.dma_start(qht[k][:, HQ:], qh[k][:, HQ:])
                    nc.sync.dma_start(qlt[k][:, HQ:], ql[k][:, HQ:])
                posct = mp.tile([128, 2 * NCOLB], dt.float32)
                nc.sync.dma_start(posct[:], posc[:, :])
                io10t = mp.tile([128, 10], dt.float32)
                nc.sync.dma_start(io10t[:], io10[:, :])

                VAL = [mp.tile([128, NCOLB], dt.float32, name=f"VAL{b}")
                       for b in range(2)]
                PRAW = [mp.tile([128, NCOLB], dt.float32, name=f"PRAW{b}")
                        for b in range(2)]
                locb = [mp.tile([128, 2 * QBLK], dt.float32, name=f"locb{b}")
                        for b in range(2)]

                def emit_half_tail(b):
                    # partition p of half b = query 512*b + 64c + p on core c
                    vi = mp.tile([64, 16], dt.float32, name=f"vi{b}")
                    nc.sync.dma_start(vi[:], glob_d[b][:].rearrange("r p e -> p r e"))
                    vals8 = vi[:, 0::2]
                    idx8 = vi[:, 1::2]
                    m8 = mp.tile([64, 1], dt.float32, name=f"m8{b}")
                    nc.vector.tensor_reduce(m8[:], vals8, mybir.AxisListType.X,
                                            _AluOp.min)
                    eq8 = mp.tile([64, 8], dt.uint8, name=f"eq8{b}")
                    nc.vector.tensor_scalar(out=eq8[:], in0=vals8,
                                            scalar1=m8[:, 0:1], scalar2=None,
                                            op0=_AluOp.is_equal)
                    big8 = mp.tile([64, 8], dt.float32, name=f"big8{b}")
                    nc.gpsimd.memset(big8[:], 1.0e9)
                    sel8 = mp.tile([64, 8], dt.float32, name=f"sel8{b}")
                    nc.vector.select(out=sel8[:], mask=eq8[:], on_true=idx8,
                                     on_false=big8[:])
                    closf = mp.tile([64, 1], dt.float32, name=f"closf{b}")
                    nc.vector.tensor_reduce(closf[:], sel8[:],
                                            mybir.AxisListType.X, _AluOp.min)
                    closi = mp.tile([64, 1], dt.int32, name=f"closi{b}")
                    nc.vector.tensor_copy(out=closi[:], in_=closf[:])
                    f2r = mp.tile([64, 10], dt.float32, name=f"f2r{b}")
                    nc.gpsimd.indirect_dma_start(
                        out=f2r[:, :], out_offset=None, in_=f2[:, :],
                        in_offset=bass.IndirectOffsetOnAxis(ap=closi[:, 0:1],
                                                            axis=0))
                    m10 = mp.tile([64, 1], dt.float32, name=f"m10{b}")
                    nc.vector.tensor_reduce(m10[:], f2r[:], mybir.AxisListType.X,
                                            _AluOp.max)
                    eqp = mp.tile([64, 10], dt.uint8, name=f"eqp{b}")
                    nc.vector.tensor_scalar(out=eqp[:], in0=f2r[:],
                                            scalar1=m10[:, 0:1], scalar2=None,
                                            op0=_AluOp.is_equal)
                    big10 = mp.tile([64, 10], dt.float32, name=f"big10{b}")
                    nc.gpsimd.memset(big10[:], 1.0e9)
                    candp = mp.tile([64, 10], dt.float32, name=f"candp{b}")
                    nc.vector.select(out=candp[:], mask=eqp[:],
                                     on_true=io10t[0:64, :], on_false=big10[:])
                    pred = mp.tile([64, 1], dt.float32, name=f"pred{b}")
                    nc.vector.tensor_reduce(pred[:], candp[:],
                                            mybir.AxisListType.X, _AluOp.min)
                    cmask = mp.tile([64, 10], dt.uint8, name=f"cmask{b}")
                    nc.vector.tensor_scalar(out=cmask[:], in0=io10t[0:64, :],
                                            scalar1=pred[:, 0:1], scalar2=None,
                                            op0=_AluOp.is_equal)
                    cmf = mp.tile([64, 10], dt.float32, name=f"cmf{b}")
                    nc.vector.tensor_copy(out=cmf[:], in_=cmask[:])
                    credst = mp.tile([64, 10], dt.float32, name=f"credst{b}")
                    nc.vector.tensor_scalar(out=credst[:], in0=cmf[:],
                                            scalar1=m10[:, 0:1], scalar2=None,
                                            op0=_AluOp.mult)
                    nc.sync.dma_start(creds_out[b * 64:(b + 1) * 64, :],
                                      credst[:])

                # ===== main loop: 2 query blocks x 13 windows =====
                for blk in range(2):
                    for w in range(NWIN):
                        off = w * WIN
                        Wc = min(WIN, SHARD_PAD - off)
                        nh = Wc // 512
                        if blk == 0 and w == 0:
                            xht, xlt = xht0, xlt0
                        else:
                            xht = [mp2.tile([128, WIN], dt.bfloat16, tag=f"xht{k}",
                                            bufs=3, name=f"xht{k}_{blk}_{w}")
                                   for k in range(2)]
                            xlt = [mp2.tile([128, WIN], dt.bfloat16, tag=f"xlt{k}",
                                            bufs=3, name=f"xlt{k}_{blk}_{w}")
                                   for k in range(2)]
                            for k in range(2):
                                nc.sync.dma_start(xht[k][:, 0:Wc],
                                                  xh[k][:, off:off + Wc])
                                nc.sync.dma_start(xlt[k][:, 0:Wc],
                                                  xl[k][:, off:off + Wc])
                        for tl in range(QBLK):
                            t = blk * QBLK + tl
                            ps = pp.tile([128, WIN], dt.float32, tag="ps", bufs=3,
                                         name=f"ps{blk}_{w}_{tl}")
                            terms = [(qht, xht), (qht, xlt), (qlt, xht)]
                            for nmm, (lhs, rhs) in enumerate(terms):
                                for k in range(2):
                                    for h in range(nh):
                                        mm = nc.tensor.matmul(
                                            ps[:, h * 512:(h + 1) * 512],
                                            lhs[k][:, t * 128:(t + 1) * 128],
                                            rhs[k][:, h * 512:(h + 1) * 512],
                                            start=(nmm == 0 and k == 0),
                                            stop=(nmm == 2 and k == 1))
                                        if h > 0:
                                            # same stationary weight as h=0:
                                            # skip the PE weight reload
                                            mm.ins.ldweights = False
                            col = w * QBLK + tl
                            scr = mp2.tile([128, WIN], dt.uint16, tag="scr",
                                           name=f"scr{blk}_{w}_{tl}")
                            nc.vector._custom_dve(
                                IDX_SCAN,
                                out=scr[:, 0:Wc][:, ::-1],
                                in0=ps[:, 0:Wc][:, ::-1],
                                in1=ssb[:, off:off + Wc][:, ::-1],
                                s0=3.4e38,
                                accum_out=PRAW[blk][:, col:col + 1])
                            jnk = mp2.tile([128, WIN], dt.uint16, tag="jnk",
                                           name=f"jnk{blk}_{w}_{tl}")
                            nc.vector._custom_dve(
                                VAL_MIN,
                                out=jnk[:, 0:Wc],
                                in0=ps[:, 0:Wc],
                                in1=ssb[:, off:off + Wc],
                                s0=3.4e38,
                                accum_out=VAL[blk][:, col:col + 1])
                        if blk == 1 and w == 10:
                            # block A's cross-core tail, hidden under block B
                            emit_half_tail(0)

                    # ===== per-block combine + collective =====
                    POSG = mp.tile([128, NCOLB], dt.float32, name=f"POSG{blk}")
                    nc.vector.tensor_tensor(
                        out=POSG[:], in0=posct[:, blk * NCOLB:(blk + 1) * NCOLB],
                        in1=PRAW[blk][:], op=_AluOp.subtract)
                    vview = VAL[blk][:].rearrange("p (s q) -> p q s", q=QBLK)
                    gmin = mp.tile([128, QBLK], dt.float32, name=f"gmin{blk}")
                    nc.vector.tensor_reduce(gmin[:], vview, mybir.AxisListType.X,
                                            _AluOp.min)
                    eqv = mp.tile([128, NCOLB], dt.uint8, name=f"eqv{blk}")
                    nc.vector.tensor_tensor(
                        out=eqv[:].rearrange("p (s q) -> p q s", q=QBLK),
                        in0=vview,
                        in1=gmin[:].unsqueeze(2).to_broadcast([128, QBLK, NWIN]),
                        op=_AluOp.is_equal)
                    big = mp.tile([128, NCOLB], dt.float32, name=f"big{blk}")
                    nc.gpsimd.memset(big[:], 1.0e9)
                    selp = mp.tile([128, NCOLB], dt.float32, name=f"selp{blk}")
                    nc.vector.select(out=selp[:], mask=eqv[:], on_true=POSG[:],
                                     on_false=big[:])
                    gpos = mp.tile([128, QBLK], dt.float32, name=f"gpos{blk}")
                    nc.vector.tensor_reduce(
                        gpos[:], selp[:].rearrange("p (s q) -> p q s", q=QBLK),
                        mybir.AxisListType.X, _AluOp.min)
                    nc.vector.tensor_copy(out=locb[blk][:, 0::2], in_=gmin[:])
                    nc.vector.tensor_copy(out=locb[blk][:, 1::2], in_=gpos[:])
                    for tl in range(QBLK):
                        nc.sync.dma_start(loc_d[blk][tl * 128:(tl + 1) * 128, :],
                                          locb[blk][:, tl * 2:tl * 2 + 2])
                    nc.gpsimd.collective_compute(
                        "AllToAll",
                        _AluOp.bypass,
                        replica_groups=[list(range(NCORES))],
                        ins=[loc_d[blk].opt()],
                        outs=[glob_d[blk].opt()],
                    )

                # ===== block-B cross-core tail (post-collective) =====
                emit_half_tail(1)

    nc.compile()
    return nc


_NC_CACHE = None
LAST_EXEC_NS = None
LAST_RESULT = None


def _get_nc():
    global _NC_CACHE
    if _NC_CACHE is None:
        _NC_CACHE = build_kernel()
    return _NC_CACHE


def _bf16_split(a):
    import ml_dtypes
    hi = a.astype(ml_dtypes.bfloat16)
    lo = (a - hi.astype(np.float32)).astype(ml_dtypes.bfloat16)
    return np.ascontiguousarray(hi), np.ascontiguousarray(lo)


def kernel(x, X, center, train_labels, train_neighbor_index, cali_nonconformity):
    x = np.asarray(x, dtype=np.float32)
    X = np.asarray(X, dtype=np.float32)
    center = np.asarray(center, dtype=np.float32)
    tni = np.asarray(train_neighbor_index, dtype=np.int64)
    labels = np.asarray(train_labels, dtype=np.int64)
    cali = np.asarray(cali_nonconformity)

    # --- query prep: xq = -2*(x/||x|| - center), transposed, bf16 split ---
    x64 = x.astype(np.float64)
    xq = (x64 / np.linalg.norm(x64, axis=1, keepdims=True)
          - center.astype(np.float64)).astype(np.float32)
    qT = np.ascontiguousarray((-2.0 * xq).T.astype(np.float32))  # [256, 1024]
    qh_in, ql_in = [], []
    for k in range(2):
        hi, lo = _bf16_split(qT[k * 128:(k + 1) * 128])
        qh_in.append(hi)
        ql_in.append(lo)

    # --- F2 table: per-train-point conformal p-values ---
    L = labels[tni]  # [100000, 74]
    counts = np.zeros((NB_TRAIN, 10), np.int64)
    for c in range(10):
        counts[:, c] = (L == c).sum(axis=1)
    counts[np.arange(NB_TRAIN), labels] += 1
    knc = 75 - counts  # knns_not_in_class
    pos = np.searchsorted(cali, knc.ravel(), side='left').reshape(knc.shape)
    f2 = ((NB_CALI - pos).astype(np.float32) / np.float32(NB_CALI))
    f2 = np.ascontiguousarray(f2)

    io10 = np.broadcast_to(np.arange(10, dtype=np.float32), (128, 10)).copy()

    in_maps = []
    for c in range(NCORES):
        Xc = np.zeros((SHARD_PAD, D), np.float32)
        Xc[:SHARD] = X[c * SHARD:(c + 1) * SHARD]
        XcT = np.ascontiguousarray(Xc.T)  # [256, 12800]
        ss = (Xc.astype(np.float64) ** 2).sum(axis=1).astype(np.float32)
        ss[SHARD:] = 1.0e4  # fake rows never win
        posc = np.zeros((128, 2 * NCOLB), np.float32)
        for blk in range(2):
            for w in range(NWIN):
                Wc = min(WIN, SHARD_PAD - w * WIN)
                cb = blk * NCOLB + w * QBLK
                posc[:, cb:cb + QBLK] = c * SHARD + w * WIN + (Wc - 1)
        m = {
            "ssg": np.ascontiguousarray(ss[None, :]),
            "posc": posc, "f2": f2, "io10": io10,
        }
        for k in range(2):
            hi, lo = _bf16_split(XcT[k * 128:(k + 1) * 128])
            m[f"xh{k}"] = hi
            m[f"xl{k}"] = lo
            m[f"qh{k}"] = qh_in[k]
            m[f"ql{k}"] = ql_in[k]
        in_maps.append(m)

    nc = _get_nc()
    trace = os.environ.get("KTRACE") == "1"
    res = run_bass_kernel_spmd(nc, in_maps, list(range(NCORES)), trace=trace)
    global LAST_EXEC_NS, LAST_RESULT
    LAST_EXEC_NS = res.exec_time_ns
    LAST_RESULT = res
    # partition p<64 of core c holds query 64c+p; p>=64 holds 512+64c+(p-64)
    out = np.empty((NB_DATA, 10), np.float32)
    for c in range(NCORES):
        cr = res.results[c]["creds"]
        out[64 * c:64 * c + 64] = cr[0:64]
        out[512 + 64 * c:512 + 64 * c + 64] = cr[64:128]
    return out


# revision 13
# speedup vs baseline: 1.0029x; 1.0029x over previous
"""DkNN retrieval kernel for 8 trn2 NeuronCores (self-contained).

Algorithm (matches reference.py):
  xq = x/||x|| - center;  score_j = ||X_j||^2 - 2 xq.X_j;  closest = argmin_j
  neigh = [closest, tni[closest]];  counts = bincount(labels[neigh]);
  p = (1000 - bisect_left(cali, 75-counts))/1000;  creds = onehot(argmax p)*max p

Distribution: X sharded over 8 cores on the train axis (12500 rows each,
padded to 12800 with fake rows whose ||X||^2 = 1e4, never winning). Queries
replicated. Matmuls use a 3-term bf16 split (hi*Hi + hi*Lo + lo*Hi) for
~2e-7 score accuracy (the rel-err gate effectively requires zero argmin
flips; one flip costs ~4e-2 rel err).

Structure: scores accumulate into 1024-wide (2-bank) PSUM windows; the two
512-col halves of a window share the stationary query weight back-to-back
(avoids the ~47ns PE weight-switch bubble on half the matmuls). Two custom
DVE ops reduce each window straight out of PSUM (argmin-position scan +
min-value accum over ps+ss). Query tiles are processed in two blocks of 4;
block A's cross-core AllToAll runs in the shadow of block B's matmuls.
Everything query-independent (||X||^2, bf16 splits, query normalization,
neighbor-label bincounts + conformal LUT folded into a per-train-point
p-value table F2[j,c]) is host preprocessing; the tail is one indirect
row-gather of F2 + a short argmax chain.
"""
import os
import numpy as np

import concourse.bass as bass
import concourse.bacc as bacc
import concourse.tile as tile
import concourse.mybir as mybir
import concourse.dve_ops as dve_ops_mod
from concourse.bass_utils import run_bass_kernel_spmd
from concourse.dve_ops import DveOp, OPS
from concourse.dve_spec import Spec, Src0, Src1, C0, MaxNeg, scan, select, eq, Idx, lower
from concourse.dve_uop import DveOpSpec, AluOp
from concourse.dve_table_gen import dve_ver_for

NB_DATA = 1024
NB_TRAIN = 100000
D = 256
NB_CALI = 1000
NCORES = 8

SHARD = 12500          # real candidates per core
SHARD_PAD = 12800      # padded (fake rows score 1e4, never win)
WIN = 1024             # candidate columns per PSUM window (2 banks)
NWIN = 13              # 12 full windows + 1 half (512)
QT = 8                 # query tiles of 128
QBLK = 4               # query tiles per collective block
NCOLB = NWIN * QBLK    # 52 accumulator columns per block

_AluOp = mybir.AluOpType


def _register_op(name, spec_fn):
    if name in dve_ops_mod._SUB_OPCODE_FOR_NAME:
        for op in OPS:
            if op.name == name:
                return op
    spec = spec_fn()
    opcode = dve_ops_mod._CUSTOM_DVE_ROW_BASE + len(OPS)
    dve_ops_mod._SUB_OPCODE_FOR_NAME[name] = opcode
    ver = dve_ver_for("TRN2")
    tmp = DveOpSpec(name=name, opcode=opcode, uops=lower(spec, ver=ver),
                    rd1_en=True)
    op = DveOp(name, spec, subdim=False, uops_sha={ver: tmp.sha(ver)})
    OPS.append(op)
    return op


def _idx_scan_spec():
    s = Src0 + Src1
    r = scan(AluOp.MIN, s, init=C0)
    body = select(eq(s, r), Idx, MaxNeg)

    def ref(in0, in1, s0, s1, imm2):
        v = (in0.astype(np.float64) + in1.astype(np.float64)).astype(np.float32)
        rm = np.minimum(np.minimum.accumulate(v, axis=-1), np.float32(s0))
        idx = np.arange(v.shape[-1], dtype=np.float64)
        sel = np.where(v == rm, idx, -3.4e38)
        return sel.astype(np.float32)

    return Spec(body=body, accum=AluOp.MAX, reference=ref)


def _val_min_spec():
    # accum_out = min over stream of (Src0 + Src1); out stream is junk
    return Spec(body=Src0 + Src1, accum=AluOp.MIN, accum_init=C0)


IDX_SCAN = _register_op("IDX_SCAN_ANT", _idx_scan_spec)
VAL_MIN = _register_op("VAL_MIN_ANT", _val_min_spec)
dt = mybir.dt


def build_kernel():
    nc = bacc.Bacc("TRN2", target_bir_lowering=False, debug=False,
                   num_devices=NCORES)

    # ---- I/O ----
    qh = [nc.dram_tensor(f"qh{k}", [128, NB_DATA], dt.bfloat16,
                         kind="ExternalInput").ap() for k in range(2)]
    ql = [nc.dram_tensor(f"ql{k}", [128, NB_DATA], dt.bfloat16,
                         kind="ExternalInput").ap() for k in range(2)]
    # packed per-window [hi(Wc) | lo(Wc)] table slices, window stride 2*WIN
    xp = [nc.dram_tensor(f"xp{k}", [128, 2 * SHARD_PAD], dt.bfloat16,
                         kind="ExternalInput").ap() for k in range(2)]
    ssg = nc.dram_tensor("ssg", [1, SHARD_PAD], dt.float32, kind="ExternalInput").ap()
    posc = nc.dram_tensor("posc", [128, 2 * NCOLB], dt.float32, kind="ExternalInput").ap()
    f2 = nc.dram_tensor("f2", [NB_TRAIN, 10], dt.float32, kind="ExternalInput").ap()
    io10 = nc.dram_tensor("io10", [128, 10], dt.float32, kind="ExternalInput").ap()
    creds_out = nc.dram_tensor("creds", [128, 10], dt.float32, kind="ExternalOutput").ap()

    with tile.TileContext(nc) as tc:
        with tc.tile_pool(name="dram", bufs=1, space="DRAM") as dpool:
            loc_d = [dpool.tile([NB_DATA // 2, 2], dt.float32, name=f"loc{b}")
                     for b in range(2)]
            glob_d = [dpool.tile([NCORES, 64, 2], dt.float32, name=f"glob{b}")
                      for b in range(2)]

            with tc.tile_pool(name="mp", bufs=1, side="right") as mp, \
                 tc.tile_pool(name="mp2", bufs=2, side="right") as mp2, \
                 tc.tile_pool(name="pp", bufs=1, space="PSUM") as pp:

                # ===== window-DMA lookahead machinery =====
                NG = 2 * NWIN  # 26 global windows (block-major)
                LOOK = 5       # windows prefetched ahead (pre-collective cover)
                wtiles = {}

                def emit_wdma(g):
                    if g >= NG:
                        return
                    w = g % NWIN
                    Wc = min(WIN, SHARD_PAD - w * WIN)
                    xpt = [mp2.tile([128, 2 * WIN], dt.bfloat16, tag=f"xp{k}",
                                    bufs=LOOK + 1, name=f"xp{k}_g{g}")
                           for k in range(2)]
                    for k in range(2):
                        nc.sync.dma_start(
                            xpt[k][:, 0:2 * Wc],
                            xp[k][:, w * 2 * WIN:w * 2 * WIN + 2 * Wc])
                    wtiles[g] = xpt

                # ===== persistent loads (critical-path DMAs first) =====
                emit_wdma(0)
                qht = [mp.tile([128, NB_DATA], dt.bfloat16, name=f"qht{k}")
                       for k in range(2)]
                qlt = [mp.tile([128, NB_DATA], dt.bfloat16, name=f"qlt{k}")
                       for k in range(2)]
                for k in range(2):
                    nc.sync.dma_start(qht[k][:], qh[k][:, :])
                    nc.sync.dma_start(qlt[k][:], ql[k][:, :])
                ssrow = mp.tile([1, SHARD_PAD], dt.float32)
                nc.sync.dma_start(ssrow[:], ssg[:, :])
                for g in range(1, LOOK):
                    emit_wdma(g)
                ssb = mp.tile([128, SHARD_PAD], dt.float32)
                CH = SHARD_PAD // 4
                for j in range(4):
                    nc.gpsimd.partition_broadcast(
                        ssb[:, j * CH:(j + 1) * CH], ssrow[:, j * CH:(j + 1) * CH])
                posct = mp.tile([128, 2 * NCOLB], dt.float32)
                nc.sync.dma_start(posct[:], posc[:, :])
                io10t = mp.tile([128, 10], dt.float32)
                nc.sync.dma_start(io10t[:], io10[:, :])

                VAL = [mp.tile([128, NCOLB], dt.float32, name=f"VAL{b}")
                       for b in range(2)]
                PRAW = [mp.tile([128, NCOLB], dt.float32, name=f"PRAW{b}")
                        for b in range(2)]
                locb = [mp.tile([128, 2 * QBLK], dt.float32, name=f"locb{b}")
                        for b in range(2)]

                def emit_half_tail(b):
                    # partition p of half b = query 512*b + 64c + p on core c
                    vi = mp.tile([64, 16], dt.float32, name=f"vi{b}")
                    nc.sync.dma_start(vi[:], glob_d[b][:].rearrange("r p e -> p r e"))
                    vals8 = vi[:, 0::2]
                    idx8 = vi[:, 1::2]
                    m8 = mp.tile([64, 1], dt.float32, name=f"m8{b}")
                    nc.vector.tensor_reduce(m8[:], vals8, mybir.AxisListType.X,
                                            _AluOp.min)
                    eq8 = mp.tile([64, 8], dt.uint8, name=f"eq8{b}")
                    nc.vector.tensor_scalar(out=eq8[:], in0=vals8,
                                            scalar1=m8[:, 0:1], scalar2=None,
                                            op0=_AluOp.is_equal)
                    big8 = mp.tile([64, 8], dt.float32, name=f"big8{b}")
                    nc.gpsimd.memset(big8[:], 1.0e9)
                    sel8 = mp.tile([64, 8], dt.float32, name=f"sel8{b}")
                    nc.vector.select(out=sel8[:], mask=eq8[:], on_true=idx8,
                                     on_false=big8[:])
                    closf = mp.tile([64, 1], dt.float32, name=f"closf{b}")
                    nc.vector.tensor_reduce(closf[:], sel8[:],
                                            mybir.AxisListType.X, _AluOp.min)
                    closi = mp.tile([64, 1], dt.int32, name=f"closi{b}")
                    nc.vector.tensor_copy(out=closi[:], in_=closf[:])
                    f2r = mp.tile([64, 10], dt.float32, name=f"f2r{b}")
                    nc.gpsimd.indirect_dma_start(
                        out=f2r[:, :], out_offset=None, in_=f2[:, :],
                        in_offset=bass.IndirectOffsetOnAxis(ap=closi[:, 0:1],
                                                            axis=0))
                    m10 = mp.tile([64, 1], dt.float32, name=f"m10{b}")
                    nc.vector.tensor_reduce(m10[:], f2r[:], mybir.AxisListType.X,
                                            _AluOp.max)
                    eqp = mp.tile([64, 10], dt.uint8, name=f"eqp{b}")
                    nc.vector.tensor_scalar(out=eqp[:], in0=f2r[:],
                                            scalar1=m10[:, 0:1], scalar2=None,
                                            op0=_AluOp.is_equal)
                    big10 = mp.tile([64, 10], dt.float32, name=f"big10{b}")
                    nc.gpsimd.memset(big10[:], 1.0e9)
                    candp = mp.tile([64, 10], dt.float32, name=f"candp{b}")
                    nc.vector.select(out=candp[:], mask=eqp[:],
                                     on_true=io10t[0:64, :], on_false=big10[:])
                    pred = mp.tile([64, 1], dt.float32, name=f"pred{b}")
                    nc.vector.tensor_reduce(pred[:], candp[:],
                                            mybir.AxisListType.X, _AluOp.min)
                    cmask = mp.tile([64, 10], dt.uint8, name=f"cmask{b}")
                    nc.vector.tensor_scalar(out=cmask[:], in0=io10t[0:64, :],
                                            scalar1=pred[:, 0:1], scalar2=None,
                                            op0=_AluOp.is_equal)
                    cmf = mp.tile([64, 10], dt.float32, name=f"cmf{b}")
                    nc.vector.tensor_copy(out=cmf[:], in_=cmask[:])
                    credst = mp.tile([64, 10], dt.float32, name=f"credst{b}")
                    nc.vector.tensor_scalar(out=credst[:], in0=cmf[:],
                                            scalar1=m10[:, 0:1], scalar2=None,
                                            op0=_AluOp.mult)
                    nc.sync.dma_start(creds_out[b * 64:(b + 1) * 64, :],
                                      credst[:])

                # ===== main loop: 2 query blocks x 13 windows =====
                for blk in range(2):
                    for w in range(NWIN):
                        g = blk * NWIN + w
                        emit_wdma(g + LOOK)
                        off = w * WIN
                        Wc = min(WIN, SHARD_PAD - off)
                        nh = Wc // 512
                        xpt = wtiles.pop(g)
                        for tl in range(QBLK):
                            t = blk * QBLK + tl
                            ps = pp.tile([128, WIN], dt.float32, tag="ps", bufs=3,
                                         name=f"ps{blk}_{w}_{tl}")
                            # rhs slices within packed tile: hi at [0,Wc),
                            # lo at [Wc, 2Wc)
                            terms = [(qht, 0), (qht, Wc), (qlt, 0)]
                            for nmm, (lhs, lo_off) in enumerate(terms):
                                for k in range(2):
                                    for h in range(nh):
                                        nc.tensor.matmul(
                                            ps[:, h * 512:(h + 1) * 512],
                                            lhs[k][:, t * 128:(t + 1) * 128],
                                            xpt[k][:, lo_off + h * 512:
                                                    lo_off + (h + 1) * 512],
                                            start=(nmm == 0 and k == 0),
                                            stop=(nmm == 2 and k == 1))
                            col = w * QBLK + tl
                            scr = mp2.tile([128, WIN], dt.uint16, tag="scr",
                                           name=f"scr{blk}_{w}_{tl}")
                            nc.vector._custom_dve(
                                IDX_SCAN,
                                out=scr[:, 0:Wc][:, ::-1],
                                in0=ps[:, 0:Wc][:, ::-1],
                                in1=ssb[:, off:off + Wc][:, ::-1],
                                s0=3.4e38,
                                accum_out=PRAW[blk][:, col:col + 1])
                            jnk = mp2.tile([128, WIN], dt.uint16, tag="jnk",
                                           name=f"jnk{blk}_{w}_{tl}")
                            nc.vector._custom_dve(
                                VAL_MIN,
                                out=jnk[:, 0:Wc],
                                in0=ps[:, 0:Wc],
                                in1=ssb[:, off:off + Wc],
                                s0=3.4e38,
                                accum_out=VAL[blk][:, col:col + 1])
                        if blk == 1 and w == 10:
                            # block A's cross-core tail, hidden under block B
                            emit_half_tail(0)

                    # ===== per-block combine + collective =====
                    POSG = mp.tile([128, NCOLB], dt.float32, name=f"POSG{blk}")
                    nc.vector.tensor_tensor(
                        out=POSG[:], in0=posct[:, blk * NCOLB:(blk + 1) * NCOLB],
                        in1=PRAW[blk][:], op=_AluOp.subtract)
                    vview = VAL[blk][:].rearrange("p (s q) -> p q s", q=QBLK)
                    gmin = mp.tile([128, QBLK], dt.float32, name=f"gmin{blk}")
                    nc.vector.tensor_reduce(gmin[:], vview, mybir.AxisListType.X,
                                            _AluOp.min)
                    eqv = mp.tile([128, NCOLB], dt.uint8, name=f"eqv{blk}")
                    nc.vector.tensor_tensor(
                        out=eqv[:].rearrange("p (s q) -> p q s", q=QBLK),
                        in0=vview,
                        in1=gmin[:].unsqueeze(2).to_broadcast([128, QBLK, NWIN]),
                        op=_AluOp.is_equal)
                    big = mp.tile([128, NCOLB], dt.float32, name=f"big{blk}")
                    nc.gpsimd.memset(big[:], 1.0e9)
                    selp = mp.tile([128, NCOLB], dt.float32, name=f"selp{blk}")
                    nc.vector.select(out=selp[:], mask=eqv[:], on_true=POSG[:],
                                     on_false=big[:])
                    gpos = mp.tile([128, QBLK], dt.float32, name=f"gpos{blk}")
                    nc.vector.tensor_reduce(
                        gpos[:], selp[:].rearrange("p (s q) -> p q s", q=QBLK),
                        mybir.AxisListType.X, _AluOp.min)
                    nc.vector.tensor_copy(out=locb[blk][:, 0::2], in_=gmin[:])
                    nc.vector.tensor_copy(out=locb[blk][:, 1::2], in_=gpos[:])
                    for tl in range(QBLK):
                        nc.sync.dma_start(loc_d[blk][tl * 128:(tl + 1) * 128, :],
                                          locb[blk][:, tl * 2:tl * 2 + 2])
                    nc.gpsimd.collective_compute(
                        "AllToAll",
                        _AluOp.bypass,
                        replica_groups=[list(range(NCORES))],
                        ins=[loc_d[blk].opt()],
                        outs=[glob_d[blk].opt()],
                    )

                # ===== block-B cross-core tail (post-collective) =====
                emit_half_tail(1)

    nc.compile()
    return nc


_NC_CACHE = None
LAST_EXEC_NS = None
LAST_RESULT = None


def _get_nc():
    global _NC_CACHE
    if _NC_CACHE is None:
        _NC_CACHE = build_kernel()
    return _NC_CACHE


def _bf16_split(a):
    import ml_dtypes
    hi = a.astype(ml_dtypes.bfloat16)
    lo = (a - hi.astype(np.float32)).astype(ml_dtypes.bfloat16)
    return np.ascontiguousarray(hi), np.ascontiguousarray(lo)


def kernel(x, X, center, train_labels, train_neighbor_index, cali_nonconformity):
    x = np.asarray(x, dtype=np.float32)
    X = np.asarray(X, dtype=np.float32)
    center = np.asarray(center, dtype=np.float32)
    tni = np.asarray(train_neighbor_index, dtype=np.int64)
    labels = np.asarray(train_labels, dtype=np.int64)
    cali = np.asarray(cali_nonconformity)

    # --- query prep: xq = -2*(x/||x|| - center), transposed, bf16 split ---
    x64 = x.astype(np.float64)
    xq = (x64 / np.linalg.norm(x64, axis=1, keepdims=True)
          - center.astype(np.float64)).astype(np.float32)
    qT = np.ascontiguousarray((-2.0 * xq).T.astype(np.float32))  # [256, 1024]
    qh_in, ql_in = [], []
    for k in range(2):
        hi, lo = _bf16_split(qT[k * 128:(k + 1) * 128])
        qh_in.append(hi)
        ql_in.append(lo)

    # --- F2 table: per-train-point conformal p-values ---
    L = labels[tni]  # [100000, 74]
    counts = np.zeros((NB_TRAIN, 10), np.int64)
    for c in range(10):
        counts[:, c] = (L == c).sum(axis=1)
    counts[np.arange(NB_TRAIN), labels] += 1
    knc = 75 - counts  # knns_not_in_class
    pos = np.searchsorted(cali, knc.ravel(), side='left').reshape(knc.shape)
    f2 = ((NB_CALI - pos).astype(np.float32) / np.float32(NB_CALI))
    f2 = np.ascontiguousarray(f2)

    io10 = np.broadcast_to(np.arange(10, dtype=np.float32), (128, 10)).copy()

    in_maps = []
    for c in range(NCORES):
        Xc = np.zeros((SHARD_PAD, D), np.float32)
        Xc[:SHARD] = X[c * SHARD:(c + 1) * SHARD]
        XcT = np.ascontiguousarray(Xc.T)  # [256, 12800]
        ss = (Xc.astype(np.float64) ** 2).sum(axis=1).astype(np.float32)
        ss[SHARD:] = 1.0e4  # fake rows never win
        posc = np.zeros((128, 2 * NCOLB), np.float32)
        for blk in range(2):
            for w in range(NWIN):
                Wc = min(WIN, SHARD_PAD - w * WIN)
                cb = blk * NCOLB + w * QBLK
                posc[:, cb:cb + QBLK] = c * SHARD + w * WIN + (Wc - 1)
        m = {
            "ssg": np.ascontiguousarray(ss[None, :]),
            "posc": posc, "f2": f2, "io10": io10,
        }
        for k in range(2):
            hi, lo = _bf16_split(XcT[k * 128:(k + 1) * 128])
            # pack per window w: [hi(Wc) | lo(Wc)] at offset w*2*WIN
            xpk = np.zeros((128, 2 * SHARD_PAD), hi.dtype)
            for w in range(NWIN):
                off = w * WIN
                Wc = min(WIN, SHARD_PAD - off)
                xpk[:, w * 2 * WIN:w * 2 * WIN + Wc] = hi[:, off:off + Wc]
                xpk[:, w * 2 * WIN + Wc:w * 2 * WIN + 2 * Wc] = lo[:, off:off + Wc]
            m[f"xp{k}"] = xpk
            m[f"qh{k}"] = qh_in[k]
            m[f"ql{k}"] = ql_in[k]
        in_maps.append(m)

    nc = _get_nc()
    trace = os.environ.get("KTRACE") == "1"
    res = run_bass_kernel_spmd(nc, in_maps, list(range(NCORES)), trace=trace)
    global LAST_EXEC_NS, LAST_RESULT
    LAST_EXEC_NS = res.exec_time_ns
    LAST_RESULT = res
    # partition p<64 of core c holds query 64c+p; p>=64 holds 512+64c+(p-64)
    out = np.empty((NB_DATA, 10), np.float32)
    for c in range(NCORES):
        cr = res.results[c]["creds"]
        out[64 * c:64 * c + 64] = cr[0:64]
        out[512 + 64 * c:512 + 64 * c + 64] = cr[64:128]
    return out
